# revision 1
# baseline (speedup 1.0000x reference)
"""nn_AttnA: fused QKV-proj + RMSnorm + RoPE + causal GQA attention + out-proj.

Data-parallel over the batch: core b computes batch element b (B=8 = 8 cores,
no collectives). Host pre-transposes/casts weights and x once; the device
kernel is fully self-contained per core.

Device pipeline per core (T=2048, C=512, 8 q-heads / 4 kv-heads, hd=64):
  1. QKV: fp16 matmuls, xT c-tiles stationary, fused [q|k|v] rhs -> psum [t,1024]
  2. RMS stats + rstd (ACT Ln/Exp, same table set as softmax Exp) + RoPE on DVE
     in [t, o] layout (fp16, 2x mode), v needs no prep
  3. PE transposes -> qT [d,t] per head pair; kT duplicated into both row
     halves so the pair's score matmuls row-pack (concurrent K=64 strips)
  4. per (head-pair, 512-wide q chunk): 2-kt-tile score groups -> psum
     [128,1024], causally narrowed; ACT Exp (scale=1/8 fused) -> fp16 pT;
     triangle mask on diagonal blocks; attnV col-packed (2 heads/bank) +
     ones-row matmul denominators; normalize via PE broadcast of 1/denom
  5. out-proj: yT t-slices stationary x WpT -> [t, o] fp32 -> DRAM
"""
import numpy as np
from contextlib import ExitStack

import concourse.bacc as bacc
import concourse.bass as bass
import concourse.tile as tile
from concourse import mybir
from concourse.bass_utils import run_bass_kernel_spmd
from concourse.masks import make_identity

F32 = mybir.dt.float32
F16 = mybir.dt.float16
AF = mybir.ActivationFunctionType

DIM = 512
EPS = 1.1920928955078125e-07
SCALE = 0.125  # 1/sqrt(64)
ROPE_BASE = 10000.0
N_CORES = 8


def build_kernel(T=2048, reps=1):
    """reps>1 re-emits the compute body for delta-timing benchmarks."""
    P = 128
    TT = T // 128
    QC = T // 512
    NPAIR = 4

    nc = bacc.Bacc()
    xT = nc.declare_dram_parameter("xT", [DIM, T], F16, isOutput=False)
    wqkvT = nc.declare_dram_parameter("wqkvT", [DIM, 1024], F16, isOutput=False)
    wpT = nc.declare_dram_parameter("wpT", [DIM, DIM], F16, isOutput=False)
    cosd = nc.declare_dram_parameter("cosd", [T, 32], F16, isOutput=False)
    sind = nc.declare_dram_parameter("sind", [T, 32], F16, isOutput=False)
    trid = nc.declare_dram_parameter("trid", [P, P], F16, isOutput=False)
    out = nc.declare_dram_parameter("out", [T, DIM], F32, isOutput=True)

    with tile.TileContext(nc) as tc, ExitStack() as ctx:
        consts = ctx.enter_context(tc.tile_pool(name="consts", bufs=1))
        big = ctx.enter_context(tc.tile_pool(name="big", bufs=1))
        work = ctx.enter_context(tc.tile_pool(name="work", bufs=2))
        pT_pool = ctx.enter_context(tc.tile_pool(name="pT", bufs=2))
        outp = ctx.enter_context(tc.tile_pool(name="outp", bufs=2))
        psA = ctx.enter_context(tc.tile_pool(name="psA", bufs=1, space="PSUM"))
        psB = ctx.enter_context(tc.tile_pool(name="psB", bufs=1, space="PSUM"))
        psC = ctx.enter_context(tc.tile_pool(name="psC", bufs=1, space="PSUM"))

        ident = consts.tile([P, P], F16)
        make_identity(nc, ident)
        ones16 = consts.tile([P, 64], F16)
        nc.vector.memset(ones16, 1.0)
        eps_b = consts.tile([P, 1], F32)
        nc.vector.memset(eps_b, EPS)
        tri = consts.tile([P, P], F16)
        nc.sync.dma_start(out=tri, in_=trid[:, :])
        cos_sb = consts.tile([P, TT * 32], F16)
        sin_sb = consts.tile([P, TT * 32], F16)
        nc.sync.dma_start(out=cos_sb.rearrange("p (tau i) -> p tau i", i=32),
                          in_=cosd.rearrange("(tau p) i -> p tau i", p=P))
        nc.sync.dma_start(out=sin_sb.rearrange("p (tau i) -> p tau i", i=32),
                          in_=sind.rearrange("(tau p) i -> p tau i", p=P))

        xT_sb = big.tile([P, 4, T], F16)
        wqkv_sb = big.tile([P, 4, 1024], F16)
        wp_sb = big.tile([P, 4, DIM], F16)
        for c in range(4):
            nc.sync.dma_start(out=xT_sb[:, c, :], in_=xT[c * P:(c + 1) * P, :])
            nc.sync.dma_start(out=wqkv_sb[:, c, :], in_=wqkvT[c * P:(c + 1) * P, :])
            nc.sync.dma_start(out=wp_sb[:, c, :], in_=wpT[c * P:(c + 1) * P, :])

        qT_sb = big.tile([P, NPAIR * T], F16)
        kT_sb = big.tile([P, NPAIR * T], F16)
        v_sb = big.tile([P, TT * 256], F16)
        yT_sb = big.tile([P, NPAIR * T], F16)

        def prep_ttile(tau):
            qkv_ps = psA.tile([P, 1024], F32, tag="psA", name="qkv_ps")
            for c in range(4):
                lhs = xT_sb[:, c, tau * P:(tau + 1) * P]
                nc.tensor.matmul(qkv_ps[:, 0:512], lhs, wqkv_sb[:, c, 0:512],
                                 start=(c == 0), stop=(c == 3))
                nc.tensor.matmul(qkv_ps[:, 512:1024], lhs, wqkv_sb[:, c, 512:1024],
                                 start=(c == 0), stop=(c == 3))
            qk16 = work.tile([P, 768], F16, tag="qk16")
            nc.scalar.activation(qk16, qkv_ps[:, 0:768], AF.Copy)
            nc.scalar.activation(v_sb[:, tau * 256:(tau + 1) * 256],
                                 qkv_ps[:, 768:1024], AF.Copy)
            sq16 = work.tile([P, 768], F16, tag="sq16")
            nc.vector.tensor_mul(sq16, qk16, qk16)
            ms = work.tile([P, 12], F32, tag="ms")
            nc.vector.tensor_reduce(ms, sq16.rearrange("p (h d) -> p h d", d=64),
                                    axis=mybir.AxisListType.X, op=mybir.AluOpType.add)
            lns = work.tile([P, 12], F32, tag="lns")
            nc.scalar.activation(lns, ms, AF.Ln, scale=1.0 / 64, bias=eps_b)
            r32 = work.tile([P, 12], F32, tag="r32")
            nc.scalar.activation(r32, lns, AF.Exp, scale=-0.5)
            qkr = work.tile([P, 768], F16, tag="qkr")
            nc.vector.tensor_mul(qkr.rearrange("p (h d) -> p h d", d=64),
                                 qk16.rearrange("p (h d) -> p h d", d=64),
                                 r32[:, :, None].broadcast_to([P, 12, 64]))
            qkrh = qkr.rearrange("p (h d) -> p h d", d=64)
            x1, x2 = qkrh[:, :, 0:32], qkrh[:, :, 32:64]
            c_b = cos_sb[:, tau * 32:(tau + 1) * 32][:, None, :].broadcast_to([P, 12, 32])
            s_b = sin_sb[:, tau * 32:(tau + 1) * 32][:, None, :].broadcast_to([P, 12, 32])
            t1 = work.tile([P, 12, 32], F16, tag="t1")
            t2 = work.tile([P, 12, 32], F16, tag="t2")
            t3 = work.tile([P, 12, 32], F16, tag="t3")
            t4 = work.tile([P, 12, 32], F16, tag="t4")
            nc.vector.tensor_mul(t1, x1, c_b)
            nc.vector.tensor_mul(t2, x2, s_b)
            nc.vector.tensor_mul(t3, x1, s_b)
            nc.vector.tensor_mul(t4, x2, c_b)
            prep = work.tile([P, 768], F16, tag="prep")
            ph = prep.rearrange("p (h d) -> p h d", d=64)
            nc.vector.tensor_add(ph[:, :, 0:32], t1, t2)
            nc.vector.tensor_sub(ph[:, :, 32:64], t4, t3)
            trk_ps = psB.tile([P, 1024], F16, tag="psB", name="trk_ps")
            for blk in range(4):
                nc.tensor.transpose(trk_ps[:, blk * P:(blk + 1) * P],
                                    prep[:, blk * P:(blk + 1) * P], ident)
            for kv in range(4):
                kin = prep[:, 512 + kv * 64: 512 + (kv + 1) * 64]
                nc.tensor.transpose(trk_ps[0:64, 512 + kv * P: 512 + (kv + 1) * P],
                                    kin, ident)
                nc.tensor.transpose(trk_ps[64:128, 512 + kv * P: 512 + (kv + 1) * P],
                                    kin, ident, tile_position=(0, 64))
            qdst = bass.AP(tensor=qT_sb.tensor, offset=qT_sb.offset + tau * P,
                           ap=[qT_sb.ap[0], [T, 4], [1, P]])
            kdst = bass.AP(tensor=kT_sb.tensor, offset=kT_sb.offset + tau * P,
                           ap=[kT_sb.ap[0], [T, 4], [1, P]])
            nc.vector.tensor_copy(qdst, trk_ps[:, 0:512].rearrange("p (g t) -> p g t", t=P))
            nc.vector.tensor_copy(kdst, trk_ps[:, 512:1024].rearrange("p (g t) -> p g t", t=P))

        def attn_pair_chunk(p, j):
            nkt = 4 * j + 4
            yT_ps = psC.tile([P, 512], F32, tag="yT", name="yT_ps")
            den_ps = psC.tile([P, 512], F32, tag="den", name="den_ps")
            for g in range(nkt // 2):
                ks = (2 * g, 2 * g + 1)
                sc_e = psA.tile([P, 1024], F32, tag="psA", name="sc_e")
                sc_o = psB.tile([P, 1024], F32, tag="psB", name="sc_o")
                for m2, k in enumerate(ks):
                    offs = max(0, P * (k - 4 * j))
                    kcol = p * T + k * P
                    qcol = p * T + 512 * j + offs
                    n = 512 - offs
                    nc.tensor.matmul(sc_e[:, 512 * m2 + offs: 512 * (m2 + 1)],
                                     kT_sb[0:64, kcol:kcol + P],
                                     qT_sb[0:64, qcol:qcol + n],
                                     start=True, stop=True)
                    nc.tensor.matmul(sc_o[:, 512 * m2 + offs: 512 * (m2 + 1)],
                                     kT_sb[64:128, kcol:kcol + P],
                                     qT_sb[64:128, qcol:qcol + n],
                                     start=True, stop=True, tile_position=(64, 0))
                pT_e = pT_pool.tile([P, 1024], F16, tag="pT_e")
                pT_o = pT_pool.tile([P, 1024], F16, tag="pT_o")
                offs0 = max(0, P * (ks[0] - 4 * j))
                offs1 = max(0, P * (ks[1] - 4 * j))
                if offs0 == 0 and offs1 == 0:
                    nc.scalar.activation(pT_e, sc_e, AF.Exp, scale=SCALE)
                    nc.scalar.activation(pT_o, sc_o, AF.Exp, scale=SCALE)
                else:
                    for m2, k in enumerate(ks):
                        offs = max(0, P * (k - 4 * j))
                        sl = slice(512 * m2 + offs, 512 * (m2 + 1))
                        nc.scalar.activation(pT_e[:, sl], sc_e[:, sl], AF.Exp, scale=SCALE)
                        nc.scalar.activation(pT_o[:, sl], sc_o[:, sl], AF.Exp, scale=SCALE)
                for m2, k in enumerate(ks):
                    if k >= 4 * j:
                        offs = P * (k - 4 * j)
                        sl = slice(512 * m2 + offs, 512 * m2 + offs + P)
                        nc.vector.tensor_mul(pT_e[:, sl], pT_e[:, sl], tri)
                        nc.vector.tensor_mul(pT_o[:, sl], pT_o[:, sl], tri)
                for m2, k in enumerate(ks):
                    offs = max(0, P * (k - 4 * j))
                    vsl = v_sb[:, k * 256 + p * 64: k * 256 + p * 64 + 64]
                    st, sp = (k == 0), (k == nkt - 1)
                    pe = pT_e[:, 512 * m2 + offs: 512 * (m2 + 1)]
                    po = pT_o[:, 512 * m2 + offs: 512 * (m2 + 1)]
                    # has_written clear is per-partition (HW-verified): the
                    # even/odd chains are independent groups. The sim tracker
                    # ignores partition offsets -> skip its check on odd.
                    nc.tensor.matmul(yT_ps[0:64, offs:512], vsl, pe,
                                     start=st, stop=sp)
                    nc.tensor.matmul(yT_ps[64:128, offs:512], vsl, po,
                                     start=st, stop=sp, tile_position=(0, 64),
                                     skip_group_check=True)
                    nc.tensor.matmul(den_ps[0:1, offs:512], ones16[:, 0:1], pe,
                                     start=st, stop=sp)
                    nc.tensor.matmul(den_ps[32:33, offs:512], ones16[:, 0:1], po,
                                     start=st, stop=sp, tile_position=(0, 32),
                                     skip_group_check=True)
            rd16 = outp.tile([P, 512], F16, tag="rd16")
            with nc.allow_low_precision(reason="fp16 denominators are plenty"):
                nc.vector.reciprocal(rd16[0:1, :], den_ps[0:1, :])
                nc.vector.reciprocal(rd16[32:33, :], den_ps[32:33, :])
            rb_ps = psC.tile([P, 512], F32, tag="den", name="rb_ps")
            nc.tensor.matmul(rb_ps[0:64, :], ones16[0:1, 0:64], rd16[0:1, :],
                             start=True, stop=True)
            nc.tensor.matmul(rb_ps[64:128, :], ones16[32:33, 0:64], rd16[32:33, :],
                             start=True, stop=True, tile_position=(32, 64),
                             skip_group_check=True)
            rb16 = outp.tile([P, 512], F16, tag="rb16")
            nc.vector.tensor_copy(rb16, rb_ps)
            nc.vector.tensor_mul(yT_sb[:, p * T + 512 * j: p * T + 512 * (j + 1)],
                                 yT_ps, rb16)

        def outproj_ttile(u):
            op_ps = psC.tile([P, 512], F32, tag="op", bufs=2, name="op_ps")
            for pair in range(4):
                nc.tensor.matmul(op_ps,
                                 yT_sb[:, pair * T + u * P: pair * T + (u + 1) * P],
                                 wp_sb[:, pair, :], start=(pair == 0), stop=(pair == 3))
            o32 = outp.tile([P, 512], F32, tag="o32")
            nc.scalar.activation(o32, op_ps, AF.Copy)
            nc.sync.dma_start(out=out[u * P:(u + 1) * P, :], in_=o32)

        for _rep in range(reps):
            for j in range(QC):
                for tau in range(4 * j, 4 * j + 4):
                    prep_ttile(tau)
                for p in range(NPAIR):
                    attn_pair_chunk(p, j)
                for u in range(4 * j, 4 * j + 4):
                    outproj_ttile(u)

    nc.finalize()
    return nc


_NC_CACHE = {}


def _get_nc(T=2048, reps=1):
    key = (T, reps)
    if key not in _NC_CACHE:
        _NC_CACHE[key] = build_kernel(T=T, reps=reps)
    return _NC_CACHE[key]


def make_host_inputs(x_b, wqkvT, wpT, cosd, sind, trid):
    return dict(xT=np.ascontiguousarray(x_b.T).astype(np.float16),
                wqkvT=wqkvT, wpT=wpT, cosd=cosd, sind=sind, trid=trid)


def make_shared_inputs(Wq, Wk, Wv, Wp, T):
    wqkvT = np.ascontiguousarray(
        np.concatenate([Wq, Wk, Wv], 0).T).astype(np.float16)
    wpT = np.ascontiguousarray(Wp.T).astype(np.float16)
    inv = 1.0 / (ROPE_BASE ** (np.arange(0, 64, 2) / 64))
    f = np.outer(np.arange(T), inv)
    cosd = np.cos(f).astype(np.float16)
    sind = np.sin(f).astype(np.float16)
    trid = (np.arange(128)[None, :] >= np.arange(128)[:, None]).astype(np.float16)
    return wqkvT, wpT, cosd, sind, trid


def kernel(x, Wq, Wk, Wv, Wp, reps=1):
    x = np.asarray(x)
    B, T, C = x.shape
    assert (B, C) == (N_CORES, DIM)
    nc = _get_nc(T=T, reps=reps)
    shared = make_shared_inputs(np.asarray(Wq), np.asarray(Wk),
                                np.asarray(Wv), np.asarray(Wp), T)
    in_maps = [make_host_inputs(x[b], *shared) for b in range(B)]
    res = run_bass_kernel_spmd(nc, in_maps, list(range(N_CORES)))
    return np.stack([res.results[b]["out"] for b in range(B)]).astype(np.float32)


# revision 4
# speedup vs baseline: 29.6259x; 29.6259x over previous
"""nn_AttnA: fused QKV-proj + RMSnorm + RoPE + causal GQA attention + out-proj.

Data-parallel over the batch: core b computes batch element b (B=8 = 8 cores,
no collectives). Host pre-transposes/casts weights and x once; the device
kernel is fully self-contained per core.

Device pipeline per core (T=2048, C=512, 8 q-heads / 4 kv-heads, hd=64):
  1. QKV: fp16 matmuls, xT c-tiles stationary, fused [q|k|v] rhs -> psum [t,1024]
  2. RMS stats + rstd (ACT Ln/Exp, same table set as softmax Exp) + RoPE on DVE
     in [t, o] layout (fp16, 2x mode), v needs no prep
  3. PE transposes -> qT [d,t] per head pair; kT duplicated into both row
     halves so the pair's score matmuls row-pack (concurrent K=64 strips)
  4. per (head-pair, 512-wide q chunk): 2-kt-tile score groups -> psum
     [128,1024], causally narrowed; ACT Exp (scale=1/8 fused) -> fp16 pT;
     triangle mask on diagonal blocks; attnV col-packed (2 heads/bank) +
     ones-row matmul denominators; normalize via PE broadcast of 1/denom
  5. out-proj: yT t-slices stationary x WpT -> [t, o] fp32 -> DRAM
"""
import numpy as np
from contextlib import ExitStack

import concourse.bacc as bacc
import concourse.bass as bass
import concourse.tile as tile
from concourse import mybir
from concourse.bass_utils import run_bass_kernel_spmd
from concourse.masks import make_identity

F32 = mybir.dt.float32
F16 = mybir.dt.float16
AF = mybir.ActivationFunctionType

DIM = 512
EPS = 1.1920928955078125e-07
SCALE = 0.125  # 1/sqrt(64)
ROPE_BASE = 10000.0
N_CORES = 8


def build_kernel(T=2048, reps=1, variant="full"):
    """reps>1 re-emits the compute body for delta-timing benchmarks.
    variant: full | qkv | prep | scores_exp | attn (bench-only strips)."""
    P = 128
    TT = T // 128
    QC = T // 512
    NPAIR = 4

    nc = bacc.Bacc()
    xT = nc.declare_dram_parameter("xT", [DIM, T], F16, isOutput=False)
    wqkvT = nc.declare_dram_parameter("wqkvT", [DIM, 1024], F16, isOutput=False)
    wpT = nc.declare_dram_parameter("wpT", [DIM, DIM], F16, isOutput=False)
    cosd = nc.declare_dram_parameter("cosd", [T, 32], F16, isOutput=False)
    sind = nc.declare_dram_parameter("sind", [T, 32], F16, isOutput=False)
    trid = nc.declare_dram_parameter("trid", [P, P], F16, isOutput=False)
    out = nc.declare_dram_parameter("out", [T, DIM], F32, isOutput=True)

    with tile.TileContext(nc) as tc, ExitStack() as ctx:
        consts = ctx.enter_context(tc.tile_pool(name="consts", bufs=1))
        big = ctx.enter_context(tc.tile_pool(name="big", bufs=1))
        work = ctx.enter_context(tc.tile_pool(name="work", bufs=2))
        pT_pool = ctx.enter_context(tc.tile_pool(name="pT", bufs=2))
        outp = ctx.enter_context(tc.tile_pool(name="outp", bufs=2))
        psA = ctx.enter_context(tc.tile_pool(name="psA", bufs=1, space="PSUM"))
        psB = ctx.enter_context(tc.tile_pool(name="psB", bufs=1, space="PSUM"))
        psC = ctx.enter_context(tc.tile_pool(name="psC", bufs=1, space="PSUM"))

        ident = consts.tile([P, P], F16)
        make_identity(nc, ident)
        ones16 = consts.tile([P, 64], F16)
        nc.vector.memset(ones16, 1.0)
        eps_b = consts.tile([P, 1], F32)
        nc.vector.memset(eps_b, EPS)
        tri = consts.tile([P, P], F16)
        nc.sync.dma_start(out=tri, in_=trid[:, :])
        cos_sb = consts.tile([P, TT * 32], F16)
        sin_sb = consts.tile([P, TT * 32], F16)
        nc.sync.dma_start(out=cos_sb.rearrange("p (tau i) -> p tau i", i=32),
                          in_=cosd.rearrange("(tau p) i -> p tau i", p=P))
        nc.sync.dma_start(out=sin_sb.rearrange("p (tau i) -> p tau i", i=32),
                          in_=sind.rearrange("(tau p) i -> p tau i", p=P))

        xT_sb = big.tile([P, 4, T], F16)
        wqkv_sb = big.tile([P, 4, 1024], F16)
        wp_sb = big.tile([P, 4, DIM], F16)
        for c in range(4):
            nc.sync.dma_start(out=xT_sb[:, c, :], in_=xT[c * P:(c + 1) * P, :])
            nc.sync.dma_start(out=wqkv_sb[:, c, :], in_=wqkvT[c * P:(c + 1) * P, :])
            nc.sync.dma_start(out=wp_sb[:, c, :], in_=wpT[c * P:(c + 1) * P, :])

        qT_sb = big.tile([P, NPAIR * T], F16)
        kT_sb = big.tile([P, NPAIR * T], F16)
        v_sb = big.tile([P, TT * 256], F16)
        yT_sb = big.tile([P, NPAIR * T], F16)

        def prep_ttile(tau):
            qkv_ps = psA.tile([P, 1024], F32, tag="psA", name="qkv_ps")
            for c in range(4):
                lhs = xT_sb[:, c, tau * P:(tau + 1) * P]
                nc.tensor.matmul(qkv_ps[:, 0:512], lhs, wqkv_sb[:, c, 0:512],
                                 start=(c == 0), stop=(c == 3))
                nc.tensor.matmul(qkv_ps[:, 512:1024], lhs, wqkv_sb[:, c, 512:1024],
                                 start=(c == 0), stop=(c == 3))
            qk16 = work.tile([P, 768], F16, tag="qk16")
            nc.scalar.activation(qk16, qkv_ps[:, 0:768], AF.Copy)
            nc.scalar.activation(v_sb[:, tau * 256:(tau + 1) * 256],
                                 qkv_ps[:, 768:1024], AF.Copy)
            sq16 = work.tile([P, 768], F16, tag="sq16")
            nc.vector.tensor_mul(sq16, qk16, qk16)
            ms = work.tile([P, 12], F32, tag="ms")
            nc.vector.tensor_reduce(ms, sq16.rearrange("p (h d) -> p h d", d=64),
                                    axis=mybir.AxisListType.X, op=mybir.AluOpType.add)
            lns = work.tile([P, 12], F32, tag="lns")
            nc.scalar.activation(lns, ms, AF.Ln, scale=1.0 / 64, bias=eps_b)
            r32 = work.tile([P, 12], F32, tag="r32")
            nc.scalar.activation(r32, lns, AF.Exp, scale=-0.5)
            qkr = work.tile([P, 768], F16, tag="qkr")
            nc.vector.tensor_mul(qkr.rearrange("p (h d) -> p h d", d=64),
                                 qk16.rearrange("p (h d) -> p h d", d=64),
                                 r32[:, :, None].broadcast_to([P, 12, 64]))
            qkrh = qkr.rearrange("p (h d) -> p h d", d=64)
            x1, x2 = qkrh[:, :, 0:32], qkrh[:, :, 32:64]
            c_b = cos_sb[:, tau * 32:(tau + 1) * 32][:, None, :].broadcast_to([P, 12, 32])
            s_b = sin_sb[:, tau * 32:(tau + 1) * 32][:, None, :].broadcast_to([P, 12, 32])
            t1 = work.tile([P, 12, 32], F16, tag="t1")
            t2 = work.tile([P, 12, 32], F16, tag="t2")
            t3 = work.tile([P, 12, 32], F16, tag="t3")
            t4 = work.tile([P, 12, 32], F16, tag="t4")
            nc.vector.tensor_mul(t1, x1, c_b)
            nc.vector.tensor_mul(t2, x2, s_b)
            nc.vector.tensor_mul(t3, x1, s_b)
            nc.vector.tensor_mul(t4, x2, c_b)
            prep = work.tile([P, 768], F16, tag="prep")
            ph = prep.rearrange("p (h d) -> p h d", d=64)
            nc.vector.tensor_add(ph[:, :, 0:32], t1, t2)
            nc.vector.tensor_sub(ph[:, :, 32:64], t4, t3)
            trk_ps = psB.tile([P, 1024], F16, tag="psB", name="trk_ps")
            for blk in range(4):
                nc.tensor.transpose(trk_ps[:, blk * P:(blk + 1) * P],
                                    prep[:, blk * P:(blk + 1) * P], ident)
            for kv in range(4):
                kin = prep[:, 512 + kv * 64: 512 + (kv + 1) * 64]
                nc.tensor.transpose(trk_ps[0:64, 512 + kv * P: 512 + (kv + 1) * P],
                                    kin, ident)
                nc.tensor.transpose(trk_ps[64:128, 512 + kv * P: 512 + (kv + 1) * P],
                                    kin, ident, tile_position=(0, 64))
            qdst = bass.AP(tensor=qT_sb.tensor, offset=qT_sb.offset + tau * P,
                           ap=[qT_sb.ap[0], [T, 4], [1, P]])
            kdst = bass.AP(tensor=kT_sb.tensor, offset=kT_sb.offset + tau * P,
                           ap=[kT_sb.ap[0], [T, 4], [1, P]])
            nc.vector.tensor_copy(qdst, trk_ps[:, 0:512].rearrange("p (g t) -> p g t", t=P))
            nc.vector.tensor_copy(kdst, trk_ps[:, 512:1024].rearrange("p (g t) -> p g t", t=P))

        def attn_pair_chunk(p, j):
            nkt = 4 * j + 4
            yT_ps = psC.tile([P, 512], F32, tag="yT", name="yT_ps")
            den_ps = psC.tile([P, 512], F32, tag="den", name="den_ps")
            for g in range(nkt // 2):
                ks = (2 * g, 2 * g + 1)
                sc_e = psA.tile([P, 1024], F32, tag="psA", name="sc_e")
                sc_o = psB.tile([P, 1024], F32, tag="psB", name="sc_o")
                for m2, k in enumerate(ks):
                    offs = max(0, P * (k - 4 * j))
                    kcol = p * T + k * P
                    qcol = p * T + 512 * j + offs
                    n = 512 - offs
                    nc.tensor.matmul(sc_e[:, 512 * m2 + offs: 512 * (m2 + 1)],
                                     kT_sb[0:64, kcol:kcol + P],
                                     qT_sb[0:64, qcol:qcol + n],
                                     start=True, stop=True)
                    nc.tensor.matmul(sc_o[:, 512 * m2 + offs: 512 * (m2 + 1)],
                                     kT_sb[64:128, kcol:kcol + P],
                                     qT_sb[64:128, qcol:qcol + n],
                                     start=True, stop=True, tile_position=(64, 0))
                pT_e = pT_pool.tile([P, 1024], F16, tag="pT_e")
                pT_o = pT_pool.tile([P, 1024], F16, tag="pT_o")
                offs0 = max(0, P * (ks[0] - 4 * j))
                offs1 = max(0, P * (ks[1] - 4 * j))
                if offs0 == 0 and offs1 == 0:
                    nc.scalar.activation(pT_e, sc_e, AF.Exp, scale=SCALE)
                    nc.scalar.activation(pT_o, sc_o, AF.Exp, scale=SCALE)
                else:
                    for m2, k in enumerate(ks):
                        offs = max(0, P * (k - 4 * j))
                        sl = slice(512 * m2 + offs, 512 * (m2 + 1))
                        nc.scalar.activation(pT_e[:, sl], sc_e[:, sl], AF.Exp, scale=SCALE)
                        nc.scalar.activation(pT_o[:, sl], sc_o[:, sl], AF.Exp, scale=SCALE)
                for m2, k in enumerate(ks):
                    if k >= 4 * j:
                        offs = P * (k - 4 * j)
                        sl = slice(512 * m2 + offs, 512 * m2 + offs + P)
                        nc.vector.tensor_mul(pT_e[:, sl], pT_e[:, sl], tri)
                        nc.vector.tensor_mul(pT_o[:, sl], pT_o[:, sl], tri)
                for m2, k in enumerate(ks):
                    offs = max(0, P * (k - 4 * j))
                    vsl = v_sb[:, k * 256 + p * 64: k * 256 + p * 64 + 64]
                    st, sp = (k == 0), (k == nkt - 1)
                    pe = pT_e[:, 512 * m2 + offs: 512 * (m2 + 1)]
                    po = pT_o[:, 512 * m2 + offs: 512 * (m2 + 1)]
                    # has_written clear is per-partition (HW-verified): the
                    # even/odd chains are independent groups. The sim tracker
                    # ignores partition offsets -> skip its check on odd.
                    nc.tensor.matmul(yT_ps[0:64, offs:512], vsl, pe,
                                     start=st, stop=sp)
                    nc.tensor.matmul(yT_ps[64:128, offs:512], vsl, po,
                                     start=st, stop=sp, tile_position=(0, 64),
                                     skip_group_check=True)
                    nc.tensor.matmul(den_ps[0:1, offs:512], ones16[:, 0:1], pe,
                                     start=st, stop=sp)
                    nc.tensor.matmul(den_ps[32:33, offs:512], ones16[:, 0:1], po,
                                     start=st, stop=sp, tile_position=(0, 32),
                                     skip_group_check=True)
            rd16 = outp.tile([P, 512], F16, tag="rd16")
            with nc.allow_low_precision(reason="fp16 denominators are plenty"):
                nc.vector.reciprocal(rd16[0:1, :], den_ps[0:1, :])
                nc.vector.reciprocal(rd16[32:33, :], den_ps[32:33, :])
            rb_ps = psC.tile([P, 512], F32, tag="den", name="rb_ps")
            nc.tensor.matmul(rb_ps[0:64, :], ones16[0:1, 0:64], rd16[0:1, :],
                             start=True, stop=True)
            nc.tensor.matmul(rb_ps[64:128, :], ones16[32:33, 0:64], rd16[32:33, :],
                             start=True, stop=True, tile_position=(32, 64),
                             skip_group_check=True)
            rb16 = outp.tile([P, 512], F16, tag="rb16")
            nc.vector.tensor_copy(rb16, rb_ps)
            nc.vector.tensor_mul(yT_sb[:, p * T + 512 * j: p * T + 512 * (j + 1)],
                                 yT_ps, rb16)

        def outproj_ttile(u):
            op_ps = psC.tile([P, 512], F32, tag="op", bufs=2, name="op_ps")
            for pair in range(4):
                nc.tensor.matmul(op_ps,
                                 yT_sb[:, pair * T + u * P: pair * T + (u + 1) * P],
                                 wp_sb[:, pair, :], start=(pair == 0), stop=(pair == 3))
            o32 = outp.tile([P, 512], F32, tag="o32")
            nc.scalar.activation(o32, op_ps, AF.Copy)
            nc.sync.dma_start(out=out[u * P:(u + 1) * P, :], in_=o32)

        def qkv_only(tau):
            qkv_ps = psA.tile([P, 1024], F32, tag="psA", name="qkv_ps")
            for c in range(4):
                lhs = xT_sb[:, c, tau * P:(tau + 1) * P]
                nc.tensor.matmul(qkv_ps[:, 0:512], lhs, wqkv_sb[:, c, 0:512],
                                 start=(c == 0), stop=(c == 3))
                nc.tensor.matmul(qkv_ps[:, 512:1024], lhs, wqkv_sb[:, c, 512:1024],
                                 start=(c == 0), stop=(c == 3))
            qk16 = work.tile([P, 768], F16, tag="qk16")
            nc.scalar.activation(qk16, qkv_ps[:, 0:768], AF.Copy)

        def scores_exp(p, j):
            nkt = 4 * j + 4
            for g in range(nkt // 2):
                ks = (2 * g, 2 * g + 1)
                sc_e = psA.tile([P, 1024], F32, tag="psA", name="sc_e")
                sc_o = psB.tile([P, 1024], F32, tag="psB", name="sc_o")
                for m2, k in enumerate(ks):
                    offs = max(0, P * (k - 4 * j))
                    kcol = p * T + k * P
                    qcol = p * T + 512 * j + offs
                    n = 512 - offs
                    nc.tensor.matmul(sc_e[:, 512 * m2 + offs: 512 * (m2 + 1)],
                                     kT_sb[0:64, kcol:kcol + P],
                                     qT_sb[0:64, qcol:qcol + n],
                                     start=True, stop=True)
                    nc.tensor.matmul(sc_o[:, 512 * m2 + offs: 512 * (m2 + 1)],
                                     kT_sb[64:128, kcol:kcol + P],
                                     qT_sb[64:128, qcol:qcol + n],
                                     start=True, stop=True, tile_position=(64, 0))
                pT_e = pT_pool.tile([P, 1024], F16, tag="pT_e")
                pT_o = pT_pool.tile([P, 1024], F16, tag="pT_o")
                offs0 = max(0, P * (ks[0] - 4 * j))
                offs1 = max(0, P * (ks[1] - 4 * j))
                if offs0 == 0 and offs1 == 0:
                    nc.scalar.activation(pT_e, sc_e, AF.Exp, scale=SCALE)
                    nc.scalar.activation(pT_o, sc_o, AF.Exp, scale=SCALE)
                else:
                    for m2, k in enumerate(ks):
                        offs = max(0, P * (k - 4 * j))
                        sl = slice(512 * m2 + offs, 512 * (m2 + 1))
                        nc.scalar.activation(pT_e[:, sl], sc_e[:, sl], AF.Exp, scale=SCALE)
                        nc.scalar.activation(pT_o[:, sl], sc_o[:, sl], AF.Exp, scale=SCALE)

        for _rep in range(reps):
            for j in range(QC):
                if variant in ("full", "prep"):
                    for tau in range(4 * j, 4 * j + 4):
                        prep_ttile(tau)
                elif variant == "qkv":
                    for tau in range(4 * j, 4 * j + 4):
                        qkv_only(tau)
                if variant == "full":
                    for p in range(NPAIR):
                        attn_pair_chunk(p, j)
                    for u in range(4 * j, 4 * j + 4):
                        outproj_ttile(u)
                elif variant == "scores_exp":
                    for p in range(NPAIR):
                        scores_exp(p, j)
                elif variant == "attn":
                    for p in range(NPAIR):
                        attn_pair_chunk(p, j)

    nc.finalize()
    return nc


_NC_CACHE = {}


def _get_nc(T=2048, reps=1):
    key = (T, reps)
    if key not in _NC_CACHE:
        _NC_CACHE[key] = build_kernel(T=T, reps=reps)
    return _NC_CACHE[key]


def make_host_inputs(x_b, wqkvT, wpT, cosd, sind, trid):
    return dict(xT=np.ascontiguousarray(x_b.T).astype(np.float16),
                wqkvT=wqkvT, wpT=wpT, cosd=cosd, sind=sind, trid=trid)


def make_shared_inputs(Wq, Wk, Wv, Wp, T):
    wqkvT = np.ascontiguousarray(
        np.concatenate([Wq, Wk, Wv], 0).T).astype(np.float16)
    wpT = np.ascontiguousarray(Wp.T).astype(np.float16)
    inv = 1.0 / (ROPE_BASE ** (np.arange(0, 64, 2) / 64))
    f = np.outer(np.arange(T), inv)
    cosd = np.cos(f).astype(np.float16)
    sind = np.sin(f).astype(np.float16)
    trid = (np.arange(128)[None, :] >= np.arange(128)[:, None]).astype(np.float16)
    return wqkvT, wpT, cosd, sind, trid


def kernel(x, Wq, Wk, Wv, Wp, reps=1):
    x = np.asarray(x)
    B, T, C = x.shape
    assert (B, C) == (N_CORES, DIM)
    nc = _get_nc(T=T, reps=reps)
    shared = make_shared_inputs(np.asarray(Wq), np.asarray(Wk),
                                np.asarray(Wv), np.asarray(Wp), T)
    in_maps = [make_host_inputs(x[b], *shared) for b in range(B)]
    res = run_bass_kernel_spmd(nc, in_maps, list(range(N_CORES)))
    return np.stack([res.results[b]["out"] for b in range(B)]).astype(np.float32)
